# revision 4
# baseline (speedup 1.0000x reference)
"""LayerNorm-LSTMCell Bass kernel for Trainium2, data-parallel over batch on 8 NeuronCores.

Computes, per the reference nn.Module:
    gates = x @ W_i + h_prev @ W_h + b          # [B, 4H], gate order i|f|g|o
    i, f, g, o = split(gates);  i,f,o = sigmoid; g = tanh
    c = f * c_prev + i * g
    h = LayerNorm(o * tanh(c)) * ln_weight + ln_bias
Returns (h, c), both [B, H] fp32.

Sharding: batch B=16384 split 8 ways (2048 rows/core); weights replicated.

Per-core design:
  - Matmuls in bf16 (fp32 would be 4x slower on the PE), accumulated in fp32 PSUM.
    x / h_prev / W are downcast for free by SWDGE cast-DMA loads.
  - Stationary operands need feature-major layout, so x/h_prev tiles are
    transposed on-chip with the 2-byte DMA-transpose xbar (SBUF->SBUF).
  - The bias is injected into the PSUM accumulation with a K=1 matmul
    (ones[1,128] as stationary, b[1,4H] as moving operand).
  - Sigmoid/Tanh on the scalar engine (same activation-table set -> one table
    load total). LayerNorm stats via bn_stats/bn_aggr on the vector engine;
    1/sqrt(var+eps) via Newton iterations seeded by the int32 bit trick, so the
    scalar engine never has to switch to the sqrt table set.
  - ln_weight/ln_bias application runs on the otherwise idle GPSIMD engine.
"""

import numpy as np

N_CORES = 8
B, I_DIM, H = 16384, 512, 512
G4 = 4 * H  # 2048
BS = B // N_CORES  # 2048 batch rows per core
P = 128
NT = BS // P  # 16 batch tiles per core
NEWTON_GROUP = 8  # batch tiles per rsqrt batch
LN_EPS = 1e-5
RSQRT_MAGIC = 0x5F3759DF

_CACHE = {}


def _emit(nc, tc, ctx):
    import concourse.bass as bass
    import concourse.mybir as mybir

    F32, BF16, I32 = mybir.dt.float32, mybir.dt.bfloat16, mybir.dt.int32
    AF = mybir.ActivationFunctionType
    OP = mybir.AluOpType

    x_d = nc.dram_tensor("x", [BS, I_DIM], F32, kind="ExternalInput").ap()
    h_d = nc.dram_tensor("h_prev", [BS, H], F32, kind="ExternalInput").ap()
    c_d = nc.dram_tensor("c_prev", [BS, H], F32, kind="ExternalInput").ap()
    wi_d = nc.dram_tensor("W_i", [I_DIM, G4], F32, kind="ExternalInput").ap()
    wh_d = nc.dram_tensor("W_h", [H, G4], F32, kind="ExternalInput").ap()
    b_d = nc.dram_tensor("b", [G4], F32, kind="ExternalInput").ap()
    lnw_d = nc.dram_tensor("ln_weight", [H], F32, kind="ExternalInput").ap()
    lnb_d = nc.dram_tensor("ln_bias", [H], F32, kind="ExternalInput").ap()
    ho_d = nc.dram_tensor("h_out", [BS, H], F32, kind="ExternalOutput").ap()
    co_d = nc.dram_tensor("c_out", [BS, H], F32, kind="ExternalOutput").ap()

    KX = I_DIM // P  # 4 k-blocks from x
    KH = H // P      # 4 k-blocks from h_prev
    KK = KX + KH     # 8

    consts = ctx.enter_context(tc.tile_pool(name="consts", bufs=1))
    loads = ctx.enter_context(tc.tile_pool(name="loads", bufs=3))
    trans = ctx.enter_context(tc.tile_pool(name="trans", bufs=3))
    gates = ctx.enter_context(tc.tile_pool(name="gates", bufs=2))
    epi = ctx.enter_context(tc.tile_pool(name="epi", bufs=3))
    hpre_pool = ctx.enter_context(tc.tile_pool(name="hpre", bufs=NEWTON_GROUP + 2))
    stat_pool = ctx.enter_context(tc.tile_pool(name="stats", bufs=3))
    grp_pool = ctx.enter_context(tc.tile_pool(name="grp", bufs=2))
    psum = ctx.enter_context(tc.tile_pool(name="psum", bufs=2, space="PSUM"))

    # --- constants -----------------------------------------------------------
    w_all = consts.tile([P, KK, G4], BF16)  # downcast weights, k-blocks side by side
    for k in range(KX):
        nc.gpsimd.dma_start(out=w_all[:, k, :], in_=wi_d[k * P:(k + 1) * P, :])
    for k in range(KH):
        nc.gpsimd.dma_start(out=w_all[:, KX + k, :], in_=wh_d[k * P:(k + 1) * P, :])

    ones_bf = consts.tile([1, P], BF16)
    nc.vector.memset(ones_bf, 1.0)
    b_bf = consts.tile([1, G4], BF16)
    b_2d = bass.AP(tensor=b_d.tensor, offset=b_d.offset,
                   ap=[[0, 1]] + [list(a) for a in b_d.ap])
    nc.gpsimd.dma_start(out=b_bf[:], in_=b_2d)

    lnw_bc = bass.AP(tensor=lnw_d.tensor, offset=lnw_d.offset,
                     ap=[[0, P]] + [list(a) for a in lnw_d.ap])
    lnw_b = consts.tile([P, H], F32)
    nc.sync.dma_start(out=lnw_b[:], in_=lnw_bc)
    lnb_bc = bass.AP(tensor=lnb_d.tensor, offset=lnb_d.offset,
                     ap=[[0, P]] + [list(a) for a in lnb_d.ap])
    lnb_b = consts.tile([P, H], F32)
    nc.sync.dma_start(out=lnb_b[:], in_=lnb_bc)

    magic = consts.tile([P, NEWTON_GROUP], I32)
    nc.vector.memset(magic, RSQRT_MAGIC)

    # --- per-group state -----------------------------------------------------
    n_groups = (NT + NEWTON_GROUP - 1) // NEWTON_GROUP

    for g in range(n_groups):
        g_lo = g * NEWTON_GROUP
        g_sz = min(NEWTON_GROUP, NT - g_lo)
        mv_g = grp_pool.tile([P, NEWTON_GROUP, 2], F32, tag="mv_g")
        inv_g = grp_pool.tile([P, NEWTON_GROUP], F32, tag="inv_g")
        nms_g = grp_pool.tile([P, NEWTON_GROUP], F32, tag="nms_g")
        hpres = []

        for tt in range(g_sz):
            t = g_lo + tt
            rows = slice(t * P, (t + 1) * P)

            # ---- load + transpose matmul operands (bf16) --------------------
            x_bf = loads.tile([P, I_DIM], BF16, tag="x_bf")
            nc.gpsimd.dma_start(out=x_bf[:], in_=x_d[rows, :])
            h_bf = loads.tile([P, H], BF16, tag="h_bf")
            nc.gpsimd.dma_start(out=h_bf[:], in_=h_d[rows, :])

            xT = trans.tile([P, KX, P], BF16, tag="xT")
            for j in range(KX):
                nc.sync.dma_start(out=xT[:, j, :], in_=x_bf[:, j * P:(j + 1) * P],
                                  transpose=True)
            hT = trans.tile([P, KH, P], BF16, tag="hT")
            for j in range(KH):
                nc.sync.dma_start(out=hT[:, j, :], in_=h_bf[:, j * P:(j + 1) * P],
                                  transpose=True)

            # ---- gates = b + x @ W_i + h_prev @ W_h (fp32 PSUM) -------------
            G = psum.tile([P, G4], F32, tag="G")
            for n in range(4):
                ns = slice(n * H, (n + 1) * H)
                nc.tensor.matmul(G[:, ns], ones_bf[:, :], b_bf[:, ns],
                                 start=True, stop=False)
            for k in range(KK):
                lhsT = xT[:, k, :] if k < KX else hT[:, k - KX, :]
                for n in range(4):
                    ns = slice(n * H, (n + 1) * H)
                    nc.tensor.matmul(G[:, ns], lhsT, w_all[:, k, ns],
                                     start=False, stop=(k == KK - 1))

            # ---- gate nonlinearities (scalar engine, one table set) ---------
            i_s = epi.tile([P, H], F32, tag="i_s")
            nc.scalar.activation(i_s[:], G[:, 0:H], AF.Sigmoid)
            f_s = epi.tile([P, H], F32, tag="f_s")
            nc.scalar.activation(f_s[:], G[:, H:2 * H], AF.Sigmoid)
            g_t = epi.tile([P, H], F32, tag="g_t")
            nc.scalar.activation(g_t[:], G[:, 2 * H:3 * H], AF.Tanh)
            o_s = epi.tile([P, H], F32, tag="o_s")
            nc.scalar.activation(o_s[:], G[:, 3 * H:4 * H], AF.Sigmoid)

            # ---- c = f*c_prev + i*g -----------------------------------------
            cp = loads.tile([P, H], F32, tag="cp")
            nc.sync.dma_start(out=cp[:], in_=c_d[rows, :])
            tmp = epi.tile([P, H], F32, tag="tmp")
            nc.vector.tensor_mul(tmp[:], i_s[:], g_t[:])
            c1 = epi.tile([P, H], F32, tag="c1")
            nc.gpsimd.tensor_mul(c1[:], f_s[:], cp[:])
            c_sb = epi.tile([P, H], F32, tag="c_sb")
            nc.vector.tensor_add(c_sb[:], c1[:], tmp[:])
            nc.sync.dma_start(out=co_d[rows, :], in_=c_sb[:])

            # ---- h_pre = o * tanh(c);  LN stats -----------------------------
            tanh_c = epi.tile([P, H], F32, tag="tanh_c")
            nc.scalar.activation(tanh_c[:], c_sb[:], AF.Tanh)
            h_pre = hpre_pool.tile([P, H], F32, tag="h_pre")
            nc.vector.tensor_mul(h_pre[:], o_s[:], tanh_c[:])
            st = stat_pool.tile([P, 6], F32, tag="st")
            nc.vector.bn_stats(out=st[:], in_=h_pre[:])
            nc.vector.bn_aggr(out=mv_g[:, tt, :], in_=st[:])
            hpres.append((t, h_pre))

        # ---- batched 1/sqrt(var+eps) via Newton (vector engine only) --------
        mu_v = mv_g[:, 0:g_sz, 0]
        var_v = mv_g[:, 0:g_sz, 1]
        v_g = grp_pool.tile([P, NEWTON_GROUP], F32, tag="v_g")
        nc.vector.tensor_scalar_add(v_g[:, 0:g_sz], var_v, LN_EPS)
        y_i = inv_g.bitcast(I32)
        nc.vector.tensor_scalar(y_i[:, 0:g_sz], v_g[:, 0:g_sz].bitcast(I32),
                                1, None, op0=OP.logical_shift_right)
        nc.vector.tensor_sub(y_i[:, 0:g_sz], magic[:, 0:g_sz], y_i[:, 0:g_sz])
        nt1 = grp_pool.tile([P, NEWTON_GROUP], F32, tag="nt1")
        for _ in range(3):  # Newton: y = y * (1.5 - 0.5 * v * y^2)
            nc.vector.tensor_mul(nt1[:, 0:g_sz], inv_g[:, 0:g_sz], inv_g[:, 0:g_sz])
            nc.vector.tensor_mul(nt1[:, 0:g_sz], nt1[:, 0:g_sz], v_g[:, 0:g_sz])
            nc.vector.tensor_scalar(nt1[:, 0:g_sz], nt1[:, 0:g_sz], -0.5, 1.5,
                                    op0=OP.mult, op1=OP.add)
            nc.vector.tensor_mul(inv_g[:, 0:g_sz], inv_g[:, 0:g_sz], nt1[:, 0:g_sz])
        # nms = -mu * inv_std
        nc.vector.scalar_tensor_tensor(nms_g[:, 0:g_sz], mu_v, -1.0,
                                       inv_g[:, 0:g_sz], op0=OP.mult, op1=OP.mult)

        # ---- normalize + ln scale/shift + store -----------------------------
        for tt, (t, h_pre) in enumerate(hpres):
            rows = slice(t * P, (t + 1) * P)
            h_n = epi.tile([P, H], F32, tag="h_n")
            nc.scalar.activation(h_n[:], h_pre[:], AF.Identity,
                                 bias=nms_g[:, tt:tt + 1], scale=inv_g[:, tt:tt + 1])
            h1 = epi.tile([P, H], F32, tag="h1")
            nc.gpsimd.tensor_mul(h1[:], h_n[:], lnw_b[:])
            h2 = epi.tile([P, H], F32, tag="h2")
            nc.gpsimd.tensor_add(h2[:], h1[:], lnb_b[:])
            nc.sync.dma_start(out=ho_d[rows, :], in_=h2[:])


def _build():
    if "nc" in _CACHE:
        return _CACHE["nc"]
    from contextlib import ExitStack
    import concourse.tile as tile
    from concourse import bacc

    nc = bacc.Bacc("TRN2", target_bir_lowering=False, debug=False)
    with tile.TileContext(nc) as tc:
        with ExitStack() as ctx:
            _emit(nc, tc, ctx)
    nc.compile()
    _CACHE["nc"] = nc
    return nc


def kernel(x, h_prev, c_prev, W_i, W_h, b, ln_weight, ln_bias):
    from concourse.bass_utils import run_bass_kernel_spmd

    nc = _build()
    in_maps = []
    for c in range(N_CORES):
        rows = slice(c * BS, (c + 1) * BS)
        in_maps.append({
            "x": np.ascontiguousarray(x[rows], dtype=np.float32),
            "h_prev": np.ascontiguousarray(h_prev[rows], dtype=np.float32),
            "c_prev": np.ascontiguousarray(c_prev[rows], dtype=np.float32),
            "W_i": np.asarray(W_i, dtype=np.float32),
            "W_h": np.asarray(W_h, dtype=np.float32),
            "b": np.asarray(b, dtype=np.float32),
            "ln_weight": np.asarray(ln_weight, dtype=np.float32),
            "ln_bias": np.asarray(ln_bias, dtype=np.float32),
        })
    res = run_bass_kernel_spmd(nc, in_maps, list(range(N_CORES)))
    h = np.concatenate([res.results[c]["h_out"] for c in range(N_CORES)], axis=0)
    c_out = np.concatenate([res.results[c]["c_out"] for c in range(N_CORES)], axis=0)
    return h, c_out
